# revision 53
# baseline (speedup 1.0000x reference)
"""Bass/Trainium2 kernel for nn_BiDirectionalCrossAttentionLayer.

Sharding: 8 cores = batch(4) x head-group(2). Each core computes, for its
batch b and its 4 heads, the full 4-stream cross-attention + the 256 output
rows (t = hg*256 .. hg*256+255) of every stream. The reference's
"transpose(1,2) ... transpose/reshape" scramble maps output row t to
(head t//64, head-dim t%64) over all sequence positions, so a head-split of
attention is exactly an output-row split of everything after it.

Schedule: software pipeline over streams. Attention for stream i is
ACT(exp)-bound; the Wo+LN1+FFN work of stream i-1 is emitted in chunks
between the scores and attn@v of stream i's pairs, filling the PE's
exp-wait windows. attn@v is emitted operand-swapped (lhsT=exp-scores,
rhs=V) so PSUM receives the attended values already transposed to
[q, channel] — no PE transposes or PSUM->SBUF spills.

All matmuls in bf16 (fp32 accumulate); residuals/LN in fp32. LayerNorm
invstd runs on DVE (Newton rsqrt from a reciprocal seed) and gelu
pre-activations are staged to SBUF with the bias folded in, so the ACT
engine keeps its exp table resident through attention and switches to
the gelu table only once per stream.
"""

import os
import numpy as np
import ml_dtypes

import concourse.bacc as bacc
import concourse.bass as bass
import concourse.tile as tile
from concourse import mybir
from concourse.bass_utils import run_bass_kernel_spmd
from concourse.masks import make_identity

BF16 = ml_dtypes.bfloat16
F32 = np.float32

NS, B, S, E, H, HD = 4, 4, 512, 512, 8, 64
SCALE = HD ** -0.5
LN_EPS = 1e-5
P = 128
HG = 2            # head groups == cores per batch
HPC = H // HG // 2  # head-pairs per core = 2
HC = H // HG      # heads per core = 4
TG = S // HG      # output rows per core per stream = 256
TS = TG // P      # row tiles per core = 2
ET = E // P       # embedding tiles = 4
KT = S // P       # key/seq tiles = 4
FT = 4 * E // P   # ffn hidden tiles = 16
N_CORES = B * HG

AF = mybir.ActivationFunctionType
ALU = mybir.AluOpType
AX = mybir.AxisListType
DT_BF = mybir.dt.bfloat16
DT_F32 = mybir.dt.float32


def _build_program(reps=1, phases="all"):
    nc = bacc.Bacc("TRN2", target_bir_lowering=False, debug=False)

    def din(name, shape, dt=DT_BF):
        return nc.dram_tensor(name, list(shape), dt, kind="ExternalInput").ap()

    xT_d = din("xT", (NS, P, ET, S))            # xT[n,p,et,s] = x[n,b,s,et*128+p]
    x32_d = din("x32", (NS, P, TS, E), DT_F32)  # x rows t-slice
    wq_d = din("wq", (NS, P, ET, HC * HD))      # Wq[n, e, hg*256 + c]
    wk_d = din("wk", (NS, P, ET, HC * HD))
    wv_d = din("wv", (NS, P, ET, HC * HD))
    wo_d = din("wo", (NS, P, ET, E))            # Wo[n]/NS, rows e
    w1_d = din("w1", (NS, P, ET, 4 * E))
    w2_d = din("w2", (NS, P, FT, E))
    cmat_d = din("cmat", (P, NS * NS), DT_F32)  # SCALE*inter broadcast on p
    g1_d = din("g1", (NS, E), DT_F32)
    b1_d = din("b1", (NS, E), DT_F32)
    g2_d = din("g2", (NS, E), DT_F32)
    b2_d = din("b2", (NS, E), DT_F32)
    bf1_d = din("bf1", (P, NS, FT), DT_F32)     # gelu bias, partition-major
    bf2_d = din("bf2", (NS, E), DT_F32)
    out_d = nc.dram_tensor("out", [NS, P, TS, E], DT_F32, kind="ExternalOutput").ap()

    with tile.TileContext(nc) as tc:
        with tc.tile_pool(name="const", bufs=1) as const:
            identf = const.tile([P, P], DT_F32)
            make_identity(nc, identf[:])
            cmat_sb = const.tile([P, NS * NS], DT_F32)
            nc.sync.dma_start(cmat_sb[:], cmat_d[:])
            # long-lived activations
            r1 = const.tile([P, NS, TS, E], DT_F32)
            r1T = const.tile([P, NS, ET, TG], DT_BF)

            # replicated body via HW loop (reps>1 only for slope timing)
            import contextlib
            _loop = tc.For_i(0, reps, 1) if reps > 1 else contextlib.nullcontext()
            with _loop:
              # fmt: off
              f_w1 = tc.alloc_tile_pool(name="f_w1", bufs=2)
              f_w2 = tc.alloc_tile_pool(name="f_w2", bufs=1)
              scopeB = tc.alloc_tile_pool(name="scopeB", bufs=1)
              x32 = scopeB.tile([P, NS, TS, E], DT_F32)
              # att: 2 live stream slots (writer i, reader i-1); j==0 writes
              # (no accumulate), so no zero-init is needed
              att = scopeB.tile([P, 2, KT, HC * HD], DT_BF)
              scopeA = tc.alloc_tile_pool(name="scopeA", bufs=1)
              qT = scopeA.tile([P, NS, HPC, S], DT_BF)   # [d-pair rows, n, hp, q]
              kT = scopeA.tile([P, NS, HPC, S], DT_BF)
              vex = scopeA.tile([P, NS, KT, HC, HD + 1], DT_BF)
              nc.gpsimd.memset(vex[:, :, :, :, HD:HD + 1], 1.0)

              # ------------- Pools for the pipelined main loop ------------
              # PSUM budget: scores 2 tags x 2 banks + uaT 2 x 1 bank +
              # mm-out 2 x 1 bank = 8 banks.
              a_sps = tc.alloc_tile_pool(name="a_sps", bufs=1, space="PSUM")
              a_ups = tc.alloc_tile_pool(name="a_ups", bufs=2, space="PSUM")
              mmps = tc.alloc_tile_pool(name="mmps", bufs=2, space="PSUM")
              a_sb = tc.alloc_tile_pool(name="a_sb", bufs=4)
              a_sm = tc.alloc_tile_pool(name="a_sm", bufs=16)
              e_sb = tc.alloc_tile_pool(name="e_sb", bufs=2)
              sq_sb = tc.alloc_tile_pool(name="sq_sb", bufs=1)
              hT_pool = tc.alloc_tile_pool(name="hT_pool", bufs=1)

              # ------------- Phase 1: QKV as interleavable pieces ---------
              # x/wq/wk/wv DMAs first so projections start ASAP; the qk/v
              # pieces for streams 1..3 are slotted into stream 0's chunk
              # slots (which have no post-attention work yet).
              p1w = tc.alloc_tile_pool(name="p1w", bufs=1)
              xTs = p1w.tile([P, NS, ET, S], DT_BF)
              wqs = p1w.tile([P, NS, ET, HC * HD], DT_BF)
              wks = p1w.tile([P, NS, ET, HC * HD], DT_BF)
              wvs = p1w.tile([P, NS, ET, HC * HD], DT_BF)
              for n in range(NS):
                  nc.sync.dma_start(xTs[:, n], xT_d[n])
                  nc.sync.dma_start(wqs[:, n], wq_d[n])
                  nc.sync.dma_start(wks[:, n], wk_d[n])
                  nc.sync.dma_start(wvs[:, n], wv_d[n])

              def qk_proj(n):
                  for hp in range(HPC):
                      ps_q = mmps.tile([P, E], DT_F32, tag="mm")
                      for et in range(ET):
                          nc.tensor.matmul(
                              ps_q[:], wqs[:, n, et, hp * P:(hp + 1) * P],
                              xTs[:, n, et], start=(et == 0),
                              stop=(et == ET - 1))
                      nc.vector.tensor_copy(qT[:, n, hp], ps_q[:])
                      ps_k = mmps.tile([P, E], DT_F32, tag="mm")
                      for et in range(ET):
                          nc.tensor.matmul(
                              ps_k[:], wks[:, n, et, hp * P:(hp + 1) * P],
                              xTs[:, n, et], start=(et == 0),
                              stop=(et == ET - 1))
                      nc.vector.tensor_copy(kT[:, n, hp], ps_k[:])

              def v_proj(n):
                  for k2 in range(KT // 2):
                      ps_v = mmps.tile([P, E], DT_F32, tag="mm")
                      for s2 in range(2):
                          kt = k2 * 2 + s2
                          for et in range(ET):
                              nc.tensor.matmul(
                                  ps_v[:, s2 * (HC * HD):(s2 + 1) * (HC * HD)],
                                  xTs[:, n, et, kt * P:(kt + 1) * P],
                                  wvs[:, n, et], start=(et == 0),
                                  stop=(et == ET - 1))
                      nc.vector.tensor_copy(
                          vex[:, n, k2 * 2:k2 * 2 + 2, :, 0:HD],
                          ps_v[:].rearrange("p (a h d) -> p a h d", a=2, d=HD))

              # post-attention constants arrive once the QKV weights retire
              CW = {}

              def setup_cw():
                  c_w = tc.alloc_tile_pool(name="c_w", bufs=1)
                  CW['pool'] = c_w
                  wos_t = c_w.tile([P, NS, ET, E], DT_BF)
                  g1b_t = c_w.tile([P, NS, E], DT_F32)
                  b1b_t = c_w.tile([P, NS, E], DT_F32)
                  g2b_t = c_w.tile([P, NS, E], DT_F32)
                  b2b_t = c_w.tile([P, NS, E], DT_F32)
                  bf2b_t = c_w.tile([P, NS, E], DT_F32)
                  bf1v_t = c_w.tile([P, NS, FT], DT_F32)
                  nc.sync.dma_start(bf1v_t[:], bf1_d[:])
                  for n in range(NS):
                      nc.sync.dma_start(x32[:, n], x32_d[n])
                      nc.sync.dma_start(wos_t[:, n], wo_d[n])
                      nc.sync.dma_start(g1b_t[:, n],
                                        g1_d[n].partition_broadcast(P))
                      nc.sync.dma_start(b1b_t[:, n],
                                        b1_d[n].partition_broadcast(P))
                      nc.sync.dma_start(g2b_t[:, n],
                                        g2_d[n].partition_broadcast(P))
                      nc.sync.dma_start(b2b_t[:, n],
                                        b2_d[n].partition_broadcast(P))
                      nc.sync.dma_start(bf2b_t[:, n],
                                        bf2_d[n].partition_broadcast(P))
                  CW.update(wos=wos_t, g1b=g1b_t, b1b=b1b_t, g2b=g2b_t,
                            b2b=b2b_t, bf2b=bf2b_t, bf1v=bf1v_t)

              w1_tiles, w2_tiles, hT_tiles = {}, {}, {}

              def emit_scores(i, j):
                  # Heads of a pair live on disjoint PE row strips (partitions
                  # 0-63 / 64-127): back-to-back issue lets the PE run them
                  # concurrently via implicit tile_position row packing.
                  c_ap = cmat_sb[:, (i * NS + j):(i * NS + j + 1)]
                  exs = []
                  for hp in range(HPC):
                      halves = []
                      for half in range(2):
                          s1 = a_sps.tile([P, 2, S], DT_F32, tag="s1")
                          s0 = a_sps.tile([P, 2, S], DT_F32, tag="s0")
                          for k2 in range(2):
                              kt = half * 2 + k2
                              nc.tensor.matmul(
                                  s1[:, k2],
                                  kT[HD:P, j, hp, kt * P:(kt + 1) * P],
                                  qT[HD:P, i, hp], start=True, stop=True)
                              nc.tensor.matmul(
                                  s0[:, k2],
                                  kT[0:HD, j, hp, kt * P:(kt + 1) * P],
                                  qT[0:HD, i, hp], start=True, stop=True)
                          ex0 = a_sb.tile([P, 2, S], DT_BF, tag="ex0")
                          ex1 = a_sb.tile([P, 2, S], DT_BF, tag="ex1")
                          nc.scalar.activation(ex1[:], s1[:], AF.Exp, scale=c_ap)
                          nc.scalar.activation(ex0[:], s0[:], AF.Exp, scale=c_ap)
                          halves.append((ex0, ex1))
                      exs.append(halves)
                  return exs

              def emit_uaT(i, j, exs):
                  # operand-swapped attn@v: out[q, hd+1] accumulated in PSUM
                  for hp in range(HPC):
                      for sub in range(2):
                          hl = hp * 2 + sub
                          uaT = a_ups.tile([P, KT, HD + 1], DT_F32, tag="ua")
                          for qt in range(KT):
                              for kt in range(KT):
                                  nc.tensor.matmul(
                                      uaT[:, qt],
                                      exs[hp][kt // 2][sub][
                                          :, kt % 2, qt * P:(qt + 1) * P],
                                      vex[:, j, kt, hl],
                                      start=(kt == 0), stop=(kt == KT - 1))
                          r4 = a_sm.tile([P, KT], DT_F32, tag="rr")
                          nc.vector.reciprocal(r4[:], uaT[:, :, HD])
                          for qt in range(KT):
                              dst = att[:, i % 2, qt, hl * HD:(hl + 1) * HD]
                              if j == 0:
                                  nc.vector.tensor_scalar_mul(
                                      dst, uaT[:, qt, 0:HD], r4[:, qt:qt + 1])
                              else:
                                  nc.vector.scalar_tensor_tensor(
                                      out=dst, in0=uaT[:, qt, 0:HD],
                                      scalar=r4[:, qt:qt + 1], in1=dst,
                                      op0=ALU.mult, op1=ALU.add)

              def layer_norm_into(out_ap, y, gb, bb):
                  # All-DVE layernorm: invstd via Newton rsqrt (reciprocal
                  # seed + 3 iterations) so ACT keeps its exp table resident
                  # (Ln/Sqrt would each force a ~1.3us table reload between
                  # the attention exps).
                  nm = a_sm.tile([P, 1], DT_F32, tag="nm")
                  nc.vector.reduce_sum(nm[:], y[:], axis=AX.X)
                  nc.vector.tensor_scalar_mul(nm[:], nm[:], -1.0 / E)
                  xc = e_sb.tile([P, E], DT_F32, tag="xc")
                  nc.vector.tensor_scalar_add(xc[:], y[:], nm[:])
                  var = a_sm.tile([P, 1], DT_F32, tag="var")
                  sq = sq_sb.tile([P, E], DT_F32, tag="sq")
                  nc.vector.scalar_tensor_tensor(
                      out=sq[:], in0=xc[:], scalar=1.0, in1=xc[:],
                      op0=ALU.mult, op1=ALU.mult, accum_out=var[:])
                  v = a_sm.tile([P, 1], DT_F32, tag="v")
                  nc.vector.tensor_scalar(
                      out=v[:], in0=var[:], scalar1=1.0 / E, scalar2=LN_EPS,
                      op0=ALU.mult, op1=ALU.add)
                  inv = a_sm.tile([P, 1], DT_F32, tag="inv")
                  nc.vector.reciprocal(inv[:], v[:])
                  # seed y0 = (1 + 1/v)/2: <7% err for v in [0.4, 2.5]
                  nc.vector.tensor_scalar(
                      out=inv[:], in0=inv[:], scalar1=0.5, scalar2=0.5,
                      op0=ALU.mult, op1=ALU.add)
                  t = a_sm.tile([P, 1], DT_F32, tag="nt")
                  for _ in range(2):
                      nc.vector.tensor_mul(t[:], inv[:], inv[:])
                      nc.vector.tensor_mul(t[:], t[:], v[:])
                      nc.vector.tensor_scalar(
                          out=t[:], in0=t[:], scalar1=-0.5, scalar2=1.5,
                          op0=ALU.mult, op1=ALU.add)
                      nc.vector.tensor_mul(inv[:], inv[:], t[:])
                  nc.vector.scalar_tensor_tensor(
                      out=out_ap, in0=xc[:], scalar=inv[:], in1=gb,
                      op0=ALU.mult, op1=ALU.mult)
                  nc.vector.tensor_add(out_ap, out_ap, bb)

              def wo_chunk(i, ts):
                  wo_ps = mmps.tile([P, E], DT_F32, tag="mm")
                  for qt in range(KT):
                      nc.tensor.matmul(
                          wo_ps[:], att[:, i % 2, qt, ts * P:(ts + 1) * P],
                          CW['wos'][:, i, qt], start=(qt == 0), stop=(qt == KT - 1))
                  y1 = e_sb.tile([P, E], DT_F32, tag="y1")
                  nc.vector.tensor_add(y1[:], wo_ps[:], x32[:, i, ts])
                  layer_norm_into(r1[:, i, ts], y1, CW['g1b'][:, i], CW['b1b'][:, i])
                  tr = mmps.tile([P, E], DT_F32, tag="mm")
                  for et in range(ET):
                      nc.tensor.transpose(
                          tr[:, et * P:(et + 1) * P],
                          r1[:, i, ts, et * P:(et + 1) * P], identf[:])
                  nc.vector.tensor_copy(
                      r1T[:, i, :, ts * P:(ts + 1) * P],
                      tr[:].rearrange("p (a b) -> p a b", b=P))

              def ffn1_chunk(n, half):
                  # matmul then stage pre-activations to SBUF (DVE has
                  # slack); gelu runs later as one contiguous ACT block so
                  # the gelu<->exp table switches are bounded to one pair
                  # per stream.
                  w1s = w1_tiles[n][half]
                  if half == 0:
                      hp_t = hT_pool.tile([P, FT, TG], DT_BF, tag="hpre")
                      hT_tiles[n] = hp_t
                  else:
                      hp_t = hT_tiles[n]
                  for f2 in range(FT // 4):
                      h_ps = mmps.tile([P, E], DT_F32, tag="mm")
                      for s2 in range(2):
                          fs_l = f2 * 2 + s2
                          for et in range(ET):
                              nc.tensor.matmul(
                                  h_ps[:, s2 * TG:(s2 + 1) * TG],
                                  w1s[:, et, fs_l * P:(fs_l + 1) * P],
                                  r1T[:, n, et],
                                  start=(et == 0), stop=(et == ET - 1))
                      for s2 in range(2):
                          fs = half * (FT // 2) + f2 * 2 + s2
                          nc.vector.tensor_scalar_add(
                              hp_t[:, fs], h_ps[:, s2 * TG:(s2 + 1) * TG],
                              CW['bf1v'][:, n, fs:fs + 1])

              def gelu_chunk(n):
                  hp_t = hT_tiles[n]
                  hT = hT_pool.tile([P, FT, TG], DT_BF, tag="hT")
                  hT_tiles[n] = hT
                  for f4 in range(FT // 4):
                      nc.scalar.activation(
                          hT[:, f4 * 4:(f4 + 1) * 4],
                          hp_t[:, f4 * 4:(f4 + 1) * 4], AF.Gelu)

              def ffn2_chunk(n, ts):
                  w2s = w2_tiles[n]
                  hT = hT_tiles[n]
                  f2_ps = mmps.tile([P, E], DT_F32, tag="mm")
                  for ft in range(FT):
                      nc.tensor.matmul(
                          f2_ps[:], hT[:, ft, ts * P:(ts + 1) * P],
                          w2s[:, ft], start=(ft == 0), stop=(ft == FT - 1))
                  y2 = e_sb.tile([P, E], DT_F32, tag="y1")
                  nc.vector.tensor_add(y2[:], f2_ps[:], CW['bf2b'][:, n])
                  nc.vector.tensor_add(y2[:], y2[:], r1[:, n, ts])
                  out_sb = e_sb.tile([P, E], DT_F32, tag="os")
                  layer_norm_into(out_sb[:], y2, CW['g2b'][:, n], CW['b2b'][:, n])
                  nc.sync.dma_start(out_d[n][:, ts], out_sb[:])
                  if ts == TS - 1:
                      del w2_tiles[n]
                      del hT_tiles[n]

              qk_proj(0)
              v_proj(0)
              for i in range(NS):
                  w1h0 = f_w1.tile([P, ET, 2 * E], DT_BF, tag="w1s")
                  w1h1 = f_w1.tile([P, ET, 2 * E], DT_BF, tag="w1s")
                  w1h = [w1h0, w1h1]
                  for h in range(2):
                      nc.sync.dma_start(
                          w1h[h][:], w1_d[i][:, :, h * 2 * E:(h + 1) * 2 * E])
                  w1_tiles[i] = w1h
                  if i >= 1:
                      w2s = f_w2.tile([P, FT, E], DT_BF, tag="w2s")
                      nc.sync.dma_start(w2s[:], w2_d[i - 1])
                      w2_tiles[i - 1] = w2s
                  if i == 0:
                      # stream 0's slots carry the remaining QKV projections
                      chunks = [
                          lambda: qk_proj(1), lambda: v_proj(1),
                          lambda: qk_proj(2), lambda: v_proj(2),
                          lambda: qk_proj(3), lambda: v_proj(3),
                      ]
                  else:
                      n = i - 1
                      chunks = [
                          lambda n=n: wo_chunk(n, 0),
                          lambda n=n: wo_chunk(n, 1),
                          lambda n=n: ffn1_chunk(n, 0),
                          lambda n=n: ffn1_chunk(n, 1),
                          lambda n=n: gelu_chunk(n),
                          lambda n=n: ffn2_chunk(n, 0),
                          lambda n=n: ffn2_chunk(n, 1),
                      ]
                      if i == 1:
                          # give setup_cw's DMAs two slots of headroom
                          chunks = [lambda: None, lambda: None] + chunks
                  ci = 0
                  for j in range(NS):
                      if i == 0 and j == 0:
                          pair_exs = emit_scores(0, 0)
                      exs = pair_exs
                      if ci < len(chunks):
                          chunks[ci]()
                          ci += 1
                      emit_uaT(i, j, exs)
                      # next pair's scores go out before chunk B so the exp
                      # stream never waits behind chunk matmuls
                      if j + 1 < NS:
                          pair_exs = emit_scores(i, j + 1)
                      elif i + 1 < NS:
                          pair_exs = emit_scores(i + 1, 0)
                      if ci < len(chunks):
                          chunks[ci]()
                          ci += 1
                  while ci < len(chunks):
                      chunks[ci]()
                      ci += 1
                  if i == 0:
                      p1w.release()
                      setup_cw()

              # tail: last stream's post-attention work, split by output-row
              # half (ts) so the two halves pipeline Wo->LN1->FFN1->gelu->FFN2
              def ffn1_ts(n, ts, hT):
                  w1h = w1_tiles[n]
                  for f4 in range(FT // 4):
                      h_ps = mmps.tile([P, E], DT_F32, tag="mm")
                      for s4 in range(4):
                          fs = f4 * 4 + s4
                          w1s = w1h[fs // (FT // 2)]
                          fs_l = fs % (FT // 2)
                          for et in range(ET):
                              nc.tensor.matmul(
                                  h_ps[:, s4 * P:(s4 + 1) * P],
                                  w1s[:, et, fs_l * P:(fs_l + 1) * P],
                                  r1T[:, n, et, ts * P:(ts + 1) * P],
                                  start=(et == 0), stop=(et == ET - 1))
                      for s4 in range(4):
                          fs = f4 * 4 + s4
                          nc.scalar.activation(
                              hT[:, fs, ts * P:(ts + 1) * P],
                              h_ps[:, s4 * P:(s4 + 1) * P],
                              AF.Gelu, bias=CW['bf1v'][:, n, fs:fs + 1])

              n = NS - 1
              w2s = f_w2.tile([P, FT, E], DT_BF, tag="w2s")
              nc.sync.dma_start(w2s[:], w2_d[n])
              w2_tiles[n] = w2s
              hT = hT_pool.tile([P, FT, TG], DT_BF, tag="hT")
              hT_tiles[n] = hT
              wo_chunk(n, 0)
              wo_chunk(n, 1)
              ffn1_ts(n, 0, hT)
              ffn1_ts(n, 1, hT)
              ffn2_chunk(n, 0)
              ffn2_chunk(n, 1)

              CW['pool'].release()
              hT_pool.release()
              sq_sb.release()
              e_sb.release()
              a_sm.release()
              a_sb.release()
              mmps.release()
              a_ups.release()
              a_sps.release()
              scopeA.release()
              scopeB.release()
              f_w2.release()
              f_w1.release()

    nc.compile()
    return nc


_NC_CACHE = {}


def _get_nc(reps=1, phases="all"):
    key = f"nc{reps}_{phases}"
    if key not in _NC_CACHE:
        _NC_CACHE[key] = _build_program(reps, phases)
    return _NC_CACHE[key]


def _pack_inputs(x0, x1, x2, x3, Wq, Wk, Wv, Wo, bo, ln1_g, ln1_b, ln2_g, ln2_b,
                 W1, bf1, W2, bf2, inter):
    x = np.stack([np.asarray(x0), np.asarray(x1), np.asarray(x2),
                  np.asarray(x3)]).astype(F32)  # [NS,B,S,E]
    Wq, Wk, Wv, Wo = (np.asarray(a, F32) for a in (Wq, Wk, Wv, Wo))
    inputs_bo = np.asarray(bo, F32)
    W1, W2 = np.asarray(W1, F32), np.asarray(W2, F32)
    inter = np.asarray(inter, F32)

    def tile_rows(a, nt):
        # [NS, R, C] -> [NS, P, nt, C]
        return np.ascontiguousarray(
            a.reshape(NS, nt, P, a.shape[-1]).transpose(0, 2, 1, 3))

    shared = {
        "wo": tile_rows(Wo / NS, ET).astype(BF16),
        "w1": tile_rows(W1, ET).astype(BF16),
        "w2": tile_rows(W2, FT).astype(BF16),
        "cmat": np.ascontiguousarray(
            np.broadcast_to((inter * SCALE).reshape(1, NS * NS), (P, NS * NS))
        ).astype(F32),
        "g1": np.ascontiguousarray(ln1_g, dtype=F32),
        "b1": np.ascontiguousarray(ln1_b, dtype=F32),
        "g2": np.ascontiguousarray(ln2_g, dtype=F32),
        "b2": np.ascontiguousarray(ln2_b, dtype=F32),
        "bf1": np.ascontiguousarray(
            np.asarray(bf1, F32).reshape(NS, FT, P).transpose(2, 0, 1)),
        "bf2": np.ascontiguousarray(bf2, dtype=F32),
    }
    per_hg = []
    for hg in range(HG):
        cols = slice(hg * HC * HD, (hg + 1) * HC * HD)
        per_hg.append({
            "wq": tile_rows(Wq[:, :, cols], ET).astype(BF16),
            "wk": tile_rows(Wk[:, :, cols], ET).astype(BF16),
            "wv": tile_rows(Wv[:, :, cols], ET).astype(BF16),
        })
    in_maps = []
    for core in range(N_CORES):
        b, hg = core // HG, core % HG
        xb = x[:, b]  # [NS, S, E]
        xT = np.ascontiguousarray(
            xb.transpose(0, 2, 1).reshape(NS, ET, P, S).transpose(0, 2, 1, 3)
        ).astype(BF16)
        x32 = np.ascontiguousarray(
            (xb[:, hg * TG:(hg + 1) * TG] + np.asarray(
                inputs_bo)[:, None, :]).reshape(NS, TS, P, E)
            .transpose(0, 2, 1, 3).astype(F32))
        m = {"xT": xT, "x32": x32}
        m.update(shared)
        m.update(per_hg[hg])
        in_maps.append(m)
    return in_maps


def _unpack_outputs(results):
    full = np.empty((NS, B, S, E), dtype=F32)
    for core in range(N_CORES):
        b, hg = core // HG, core % HG
        o = results[core]["out"]  # [NS, P, TS, E]
        full[:, b, hg * TG:(hg + 1) * TG] = (
            o.transpose(0, 2, 1, 3).reshape(NS, TG, E))
    return tuple(full[n] for n in range(NS))


def kernel(**inputs):
    nc = _get_nc()
    in_maps = _pack_inputs(**inputs)
    res = run_bass_kernel_spmd(
        nc, in_maps, core_ids=list(range(N_CORES)),
        trace=bool(int(os.environ.get("KERNEL_TRACE", "0"))))
    _NC_CACHE["last_result"] = res
    return _unpack_outputs(res.results)


def _bench_fn(inputs, reps=1, phases="all"):
    """Build a re-invocable jitted executable for the reps-replicated NEFF.

    Mirrors bass2jax.run_bass_via_pjrt's shard_map(_bass_exec) lowering but
    without output-buffer donation, so the same executable can be re-invoked
    and timed.
    """
    import jax
    from jax.sharding import Mesh, PartitionSpec, NamedSharding
    from jax.experimental.shard_map import shard_map
    from concourse import bass2jax
    from concourse import mybir as mb

    nc = _get_nc(reps, phases)
    bass2jax.install_neuronx_cc_hook()
    in_maps = _pack_inputs(**inputs)

    part_name = nc.partition_id_tensor.name if nc.partition_id_tensor else None
    in_names, out_names, out_avals, zero_outs = [], [], [], []
    for alloc in nc.m.functions[0].allocations:
        if not isinstance(alloc, mb.MemoryLocationSet):
            continue
        name = alloc.memorylocations[0].name
        if alloc.kind == "ExternalInput":
            if name != part_name:
                in_names.append(name)
        elif alloc.kind == "ExternalOutput":
            out_names.append(name)
            shape = tuple(alloc.tensor_shape)
            dtype = mb.dt.np(alloc.dtype)
            out_avals.append(jax.core.ShapedArray(shape, dtype))
            zero_outs.append(np.zeros(shape, dtype))
    n_params = len(in_names)
    all_names = in_names + out_names
    if part_name is not None:
        all_names = all_names + [part_name]

    def _body(*args):
        operands = list(args)
        if part_name is not None:
            operands.append(bass2jax.partition_id_tensor())
        outs = bass2jax._bass_exec_p.bind(
            *operands, out_avals=tuple(out_avals), in_names=tuple(all_names),
            out_names=tuple(out_names), lowering_input_output_aliases=(),
            sim_require_finite=True, sim_require_nnan=True, nc=nc)
        return tuple(outs)

    devices = jax.devices()[:N_CORES]
    mesh = Mesh(np.asarray(devices), ("core",))
    spec = PartitionSpec("core")
    fn = jax.jit(shard_map(
        _body, mesh=mesh, in_specs=(spec,) * (n_params + len(out_names)),
        out_specs=(spec,) * len(out_names), check_rep=False))
    sh = NamedSharding(mesh, spec)
    concat = [jax.device_put(
        np.concatenate([in_maps[c][nm] for c in range(N_CORES)], axis=0), sh)
        for nm in in_names]
    concat += [jax.device_put(
        np.zeros((N_CORES * z.shape[0], *z.shape[1:]), z.dtype), sh)
        for z in zero_outs]

    return fn, concat


def bench(inputs, iters=20, reps=1, phases="all"):
    """(min, median) wall seconds per call of the reps-replicated NEFF."""
    import time
    import jax

    fn, concat = _bench_fn(inputs, reps, phases)
    out = fn(*concat)  # compile
    jax.block_until_ready(out)
    times = []
    for _ in range(iters):
        t0 = time.perf_counter()
        out = fn(*concat)
        jax.block_until_ready(out)
        times.append(time.perf_counter() - t0)
    times.sort()
    return times[0], times[len(times) // 2]


def bench_interleaved(inputs, iters=30, reps=8):
    """Per-rep device time via interleaved reps=1 / reps=R calls.

    Alternating the two executables within each round and taking the median
    of per-round deltas cancels the axon-RPC overhead drift that breaks the
    separate-min slope estimate.
    """
    import time
    import jax

    fn1, args1 = _bench_fn(inputs, 1)
    fnR, argsR = _bench_fn(inputs, reps)
    jax.block_until_ready(fn1(*args1))
    jax.block_until_ready(fnR(*argsR))
    deltas = []
    for _ in range(iters):
        t0 = time.perf_counter()
        jax.block_until_ready(fn1(*args1))
        t1 = time.perf_counter()
        jax.block_until_ready(fnR(*argsR))
        t2 = time.perf_counter()
        deltas.append((t2 - t1) - (t1 - t0))
    deltas.sort()
    med = deltas[len(deltas) // 2]
    return med / (reps - 1), deltas


if __name__ == "__main__":
    import sys
    mode = sys.argv[1] if len(sys.argv) > 1 else "sim"
    sys.path.insert(0, os.path.dirname(os.path.abspath(__file__)))
    import reference

    inputs = {k: np.asarray(v) for k, v in reference.setup_inputs().items()}
    if mode == "sim":
        # Simulate core 0 (b=0, hg=0) with CoreSim and compare to reference.
        # CoreSim has no Gelu; patch exact erf-gelu into its activation visitor.
        import concourse.bass_interp as bass_interp
        from scipy.special import erf as _erf
        _orig_visit = bass_interp.InstructionExecutor.visit_InstActivation

        def _patched(self, instruction, reg_snapshot=None):
            if instruction.func == mybir.ActivationFunctionType.Gelu:
                instruction.func = mybir.ActivationFunctionType.Identity
                try:
                    import concourse.mybir as mb
                    from concourse.bass_interp import Direction
                    out_ap = instruction.outs[0]
                    res = _orig_visit(self, instruction, reg_snapshot=reg_snapshot)
                    v = self.view_ap(out_ap, Direction.WRITE, instruction,
                                     reg_snapshot=reg_snapshot)
                    x = v[:].astype(np.float32)
                    v[:] = (x * 0.5 * (1.0 + _erf(x / np.sqrt(2.0)))).astype(v.dtype)
                    return res
                finally:
                    instruction.func = mybir.ActivationFunctionType.Gelu
            return _orig_visit(self, instruction, reg_snapshot=reg_snapshot)

        bass_interp.InstructionExecutor.visit_InstActivation = _patched
        from concourse.bass_interp import CoreSim
        nc = _get_nc()
        in_maps = _pack_inputs(**inputs)
        sim = CoreSim(nc, trace=False)
        for name, arr in in_maps[0].items():
            sim.tensor(name)[:] = arr
        sim.simulate(check_with_hw=False)
        out = sim.tensor("out").copy()
        got = out.transpose(0, 2, 1, 3).reshape(NS, TG, E)
        exp = np.stack([np.asarray(o) for o in reference.reference(**inputs)])
        exp_slice = exp[:, 0, 0:TG]  # b=0, rows 0:256
        err = np.abs(got - exp_slice)
        rel = np.linalg.norm(got - exp_slice) / np.linalg.norm(exp_slice)
        print(f"max abs err: {err.max():.3e}  rel fro err: {rel:.3e}")
    else:
        got = kernel(**inputs)
        exp = reference.reference(**inputs)
        for n in range(NS):
            g, e = np.asarray(got[n]), np.asarray(exp[n])
            rel = np.linalg.norm(g - e) / np.linalg.norm(e)
            print(f"out{n}: rel fro err {rel:.3e} max abs {np.abs(g - e).max():.3e}")


# revision 55
# speedup vs baseline: 1.0227x; 1.0227x over previous
"""Bass/Trainium2 kernel for nn_BiDirectionalCrossAttentionLayer.

Sharding: 8 cores = batch(4) x head-group(2). Each core computes, for its
batch b and its 4 heads, the full 4-stream cross-attention + the 256 output
rows (t = hg*256 .. hg*256+255) of every stream. The reference's
"transpose(1,2) ... transpose/reshape" scramble maps output row t to
(head t//64, head-dim t%64) over all sequence positions, so a head-split of
attention is exactly an output-row split of everything after it.

Schedule: software pipeline over streams. Attention for stream i is
ACT(exp)-bound; the Wo+LN1+FFN work of stream i-1 is emitted in chunks
between the scores and attn@v of stream i's pairs, filling the PE's
exp-wait windows. attn@v is emitted operand-swapped (lhsT=exp-scores,
rhs=V) so PSUM receives the attended values already transposed to
[q, channel] — no PE transposes or PSUM->SBUF spills.

All matmuls in bf16 (fp32 accumulate); residuals/LN in fp32. LayerNorm
invstd runs on DVE (Newton rsqrt from a reciprocal seed) and gelu
pre-activations are staged to SBUF with the bias folded in, so the ACT
engine keeps its exp table resident through attention and switches to
the gelu table only once per stream.
"""

import os
import numpy as np
import ml_dtypes

import concourse.bacc as bacc
import concourse.bass as bass
import concourse.tile as tile
from concourse import mybir
from concourse.bass_utils import run_bass_kernel_spmd
from concourse.masks import make_identity

BF16 = ml_dtypes.bfloat16
F32 = np.float32

NS, B, S, E, H, HD = 4, 4, 512, 512, 8, 64
SCALE = HD ** -0.5
LN_EPS = 1e-5
P = 128
HG = 2            # head groups == cores per batch
HPC = H // HG // 2  # head-pairs per core = 2
HC = H // HG      # heads per core = 4
TG = S // HG      # output rows per core per stream = 256
TS = TG // P      # row tiles per core = 2
ET = E // P       # embedding tiles = 4
KT = S // P       # key/seq tiles = 4
FT = 4 * E // P   # ffn hidden tiles = 16
N_CORES = B * HG

AF = mybir.ActivationFunctionType
ALU = mybir.AluOpType
AX = mybir.AxisListType
DT_BF = mybir.dt.bfloat16
DT_F32 = mybir.dt.float32


def _build_program(reps=1, phases="all"):
    nc = bacc.Bacc("TRN2", target_bir_lowering=False, debug=False)

    def din(name, shape, dt=DT_BF):
        return nc.dram_tensor(name, list(shape), dt, kind="ExternalInput").ap()

    xT_d = din("xT", (NS, P, ET, S))            # xT[n,p,et,s] = x[n,b,s,et*128+p]
    x32_d = din("x32", (NS, P, TS, E), DT_F32)  # x rows t-slice
    wq_d = din("wq", (NS, P, ET, HC * HD))      # Wq[n, e, hg*256 + c]
    wk_d = din("wk", (NS, P, ET, HC * HD))
    wv_d = din("wv", (NS, P, ET, HC * HD))
    wo_d = din("wo", (NS, P, ET, E))            # Wo[n]/NS, rows e
    w1_d = din("w1", (NS, P, ET, 4 * E))
    w2_d = din("w2", (NS, P, FT, E))
    cmat_d = din("cmat", (P, NS * NS), DT_F32)  # SCALE*inter broadcast on p
    g1_d = din("g1", (NS, E), DT_F32)
    b1_d = din("b1", (NS, E), DT_F32)
    g2_d = din("g2", (NS, E), DT_F32)
    b2_d = din("b2", (NS, E), DT_F32)
    bf1_d = din("bf1", (P, NS, FT), DT_F32)     # gelu bias, partition-major
    bf2_d = din("bf2", (NS, E), DT_F32)
    out_d = nc.dram_tensor("out", [NS, P, TS, E], DT_F32, kind="ExternalOutput").ap()

    with tile.TileContext(nc) as tc:
        with tc.tile_pool(name="const", bufs=1) as const:
            identf = const.tile([P, P], DT_F32)
            make_identity(nc, identf[:])
            cmat_sb = const.tile([P, NS * NS], DT_F32)
            nc.sync.dma_start(cmat_sb[:], cmat_d[:])
            eps_sb = const.tile([P, 1], DT_F32)
            nc.gpsimd.memset(eps_sb[:], LN_EPS)

            # long-lived activations
            r1 = const.tile([P, NS, TS, E], DT_F32)
            r1T = const.tile([P, NS, ET, TG], DT_BF)

            # replicated body via HW loop (reps>1 only for slope timing)
            import contextlib
            _loop = tc.For_i(0, reps, 1) if reps > 1 else contextlib.nullcontext()
            with _loop:
              # fmt: off
              f_w1 = tc.alloc_tile_pool(name="f_w1", bufs=2)
              f_w2 = tc.alloc_tile_pool(name="f_w2", bufs=1)
              scopeB = tc.alloc_tile_pool(name="scopeB", bufs=1)
              x32 = scopeB.tile([P, NS, TS, E], DT_F32)
              # att: 2 live stream slots (writer i, reader i-1); j==0 writes
              # (no accumulate), so no zero-init is needed
              att = scopeB.tile([P, 2, KT, HC * HD], DT_BF)
              scopeA = tc.alloc_tile_pool(name="scopeA", bufs=1)
              qT = scopeA.tile([P, NS, HPC, S], DT_BF)   # [d-pair rows, n, hp, q]
              kT = scopeA.tile([P, NS, HPC, S], DT_BF)
              vex = scopeA.tile([P, NS, KT, HC, HD + 1], DT_BF)
              nc.gpsimd.memset(vex[:, :, :, :, HD:HD + 1], 1.0)

              # ------------- Pools for the pipelined main loop ------------
              # PSUM budget: scores 2 tags x 2 banks + uaT 2 x 1 bank +
              # mm-out 2 x 1 bank = 8 banks.
              a_sps = tc.alloc_tile_pool(name="a_sps", bufs=1, space="PSUM")
              a_ups = tc.alloc_tile_pool(name="a_ups", bufs=2, space="PSUM")
              mmps = tc.alloc_tile_pool(name="mmps", bufs=2, space="PSUM")
              a_sb = tc.alloc_tile_pool(name="a_sb", bufs=4)
              a_sm = tc.alloc_tile_pool(name="a_sm", bufs=16)
              e_sb = tc.alloc_tile_pool(name="e_sb", bufs=2)
              sq_sb = tc.alloc_tile_pool(name="sq_sb", bufs=1)
              hT_pool = tc.alloc_tile_pool(name="hT_pool", bufs=1)

              # ------------- Phase 1: QKV as interleavable pieces ---------
              # x/wq/wk/wv DMAs first so projections start ASAP; the qk/v
              # pieces for streams 1..3 are slotted into stream 0's chunk
              # slots (which have no post-attention work yet).
              p1w = tc.alloc_tile_pool(name="p1w", bufs=1)
              xTs = p1w.tile([P, NS, ET, S], DT_BF)
              wqs = p1w.tile([P, NS, ET, HC * HD], DT_BF)
              wks = p1w.tile([P, NS, ET, HC * HD], DT_BF)
              wvs = p1w.tile([P, NS, ET, HC * HD], DT_BF)
              for n in range(NS):
                  nc.sync.dma_start(xTs[:, n], xT_d[n])
                  nc.sync.dma_start(wqs[:, n], wq_d[n])
                  nc.sync.dma_start(wks[:, n], wk_d[n])
                  nc.sync.dma_start(wvs[:, n], wv_d[n])

              def qk_proj(n):
                  for hp in range(HPC):
                      ps_q = mmps.tile([P, E], DT_F32, tag="mm")
                      for et in range(ET):
                          nc.tensor.matmul(
                              ps_q[:], wqs[:, n, et, hp * P:(hp + 1) * P],
                              xTs[:, n, et], start=(et == 0),
                              stop=(et == ET - 1))
                      nc.vector.tensor_copy(qT[:, n, hp], ps_q[:])
                      ps_k = mmps.tile([P, E], DT_F32, tag="mm")
                      for et in range(ET):
                          nc.tensor.matmul(
                              ps_k[:], wks[:, n, et, hp * P:(hp + 1) * P],
                              xTs[:, n, et], start=(et == 0),
                              stop=(et == ET - 1))
                      nc.vector.tensor_copy(kT[:, n, hp], ps_k[:])

              def v_proj(n):
                  for k2 in range(KT // 2):
                      ps_v = mmps.tile([P, E], DT_F32, tag="mm")
                      for s2 in range(2):
                          kt = k2 * 2 + s2
                          for et in range(ET):
                              nc.tensor.matmul(
                                  ps_v[:, s2 * (HC * HD):(s2 + 1) * (HC * HD)],
                                  xTs[:, n, et, kt * P:(kt + 1) * P],
                                  wvs[:, n, et], start=(et == 0),
                                  stop=(et == ET - 1))
                      nc.vector.tensor_copy(
                          vex[:, n, k2 * 2:k2 * 2 + 2, :, 0:HD],
                          ps_v[:].rearrange("p (a h d) -> p a h d", a=2, d=HD))

              # post-attention constants arrive once the QKV weights retire
              CW = {}

              def setup_cw():
                  c_w = tc.alloc_tile_pool(name="c_w", bufs=1)
                  CW['pool'] = c_w
                  wos_t = c_w.tile([P, NS, ET, E], DT_BF)
                  g1b_t = c_w.tile([P, NS, E], DT_F32)
                  b1b_t = c_w.tile([P, NS, E], DT_F32)
                  g2b_t = c_w.tile([P, NS, E], DT_F32)
                  b2b_t = c_w.tile([P, NS, E], DT_F32)
                  bf2b_t = c_w.tile([P, NS, E], DT_F32)
                  bf1v_t = c_w.tile([P, NS, FT], DT_F32)
                  nc.sync.dma_start(bf1v_t[:], bf1_d[:])
                  for n in range(NS):
                      nc.sync.dma_start(x32[:, n], x32_d[n])
                      nc.sync.dma_start(wos_t[:, n], wo_d[n])
                      nc.sync.dma_start(g1b_t[:, n],
                                        g1_d[n].partition_broadcast(P))
                      nc.sync.dma_start(b1b_t[:, n],
                                        b1_d[n].partition_broadcast(P))
                      nc.sync.dma_start(g2b_t[:, n],
                                        g2_d[n].partition_broadcast(P))
                      nc.sync.dma_start(b2b_t[:, n],
                                        b2_d[n].partition_broadcast(P))
                      nc.sync.dma_start(bf2b_t[:, n],
                                        bf2_d[n].partition_broadcast(P))
                  CW.update(wos=wos_t, g1b=g1b_t, b1b=b1b_t, g2b=g2b_t,
                            b2b=b2b_t, bf2b=bf2b_t, bf1v=bf1v_t)

              w1_tiles, w2_tiles, hT_tiles = {}, {}, {}

              def emit_scores(i, j):
                  # Heads of a pair live on disjoint PE row strips (partitions
                  # 0-63 / 64-127): back-to-back issue lets the PE run them
                  # concurrently via implicit tile_position row packing.
                  c_ap = cmat_sb[:, (i * NS + j):(i * NS + j + 1)]
                  exs = []
                  for hp in range(HPC):
                      halves = []
                      for half in range(2):
                          s1 = a_sps.tile([P, 2, S], DT_F32, tag="s1")
                          s0 = a_sps.tile([P, 2, S], DT_F32, tag="s0")
                          for k2 in range(2):
                              kt = half * 2 + k2
                              nc.tensor.matmul(
                                  s1[:, k2],
                                  kT[HD:P, j, hp, kt * P:(kt + 1) * P],
                                  qT[HD:P, i, hp], start=True, stop=True)
                              nc.tensor.matmul(
                                  s0[:, k2],
                                  kT[0:HD, j, hp, kt * P:(kt + 1) * P],
                                  qT[0:HD, i, hp], start=True, stop=True)
                          ex0 = a_sb.tile([P, 2, S], DT_BF, tag="ex0")
                          ex1 = a_sb.tile([P, 2, S], DT_BF, tag="ex1")
                          nc.scalar.activation(ex1[:], s1[:], AF.Exp, scale=c_ap)
                          nc.scalar.activation(ex0[:], s0[:], AF.Exp, scale=c_ap)
                          halves.append((ex0, ex1))
                      exs.append(halves)
                  return exs

              def emit_uaT(i, j, exs):
                  # operand-swapped attn@v: out[q, hd+1] accumulated in PSUM
                  for hp in range(HPC):
                      for sub in range(2):
                          hl = hp * 2 + sub
                          uaT = a_ups.tile([P, KT, HD + 1], DT_F32, tag="ua")
                          for qt in range(KT):
                              for kt in range(KT):
                                  nc.tensor.matmul(
                                      uaT[:, qt],
                                      exs[hp][kt // 2][sub][
                                          :, kt % 2, qt * P:(qt + 1) * P],
                                      vex[:, j, kt, hl],
                                      start=(kt == 0), stop=(kt == KT - 1))
                          r4 = a_sm.tile([P, KT], DT_F32, tag="rr")
                          nc.vector.reciprocal(r4[:], uaT[:, :, HD])
                          for qt in range(KT):
                              dst = att[:, i % 2, qt, hl * HD:(hl + 1) * HD]
                              if j == 0:
                                  nc.vector.tensor_scalar_mul(
                                      dst, uaT[:, qt, 0:HD], r4[:, qt:qt + 1])
                              else:
                                  nc.vector.scalar_tensor_tensor(
                                      out=dst, in0=uaT[:, qt, 0:HD],
                                      scalar=r4[:, qt:qt + 1], in1=dst,
                                      op0=ALU.mult, op1=ALU.add)

              def layer_norm_into(out_ap, y, gb, bb, act_ln=False):
                  # All-DVE layernorm: invstd via Newton rsqrt (reciprocal
                  # seed + 3 iterations) so ACT keeps its exp table resident
                  # (Ln/Sqrt would each force a ~1.3us table reload between
                  # the attention exps).
                  nm = a_sm.tile([P, 1], DT_F32, tag="nm")
                  nc.vector.reduce_sum(nm[:], y[:], axis=AX.X)
                  nc.vector.tensor_scalar_mul(nm[:], nm[:], -1.0 / E)
                  xc = e_sb.tile([P, E], DT_F32, tag="xc")
                  nc.vector.tensor_scalar_add(xc[:], y[:], nm[:])
                  var = a_sm.tile([P, 1], DT_F32, tag="var")
                  sq = sq_sb.tile([P, E], DT_F32, tag="sq")
                  nc.vector.scalar_tensor_tensor(
                      out=sq[:], in0=xc[:], scalar=1.0, in1=xc[:],
                      op0=ALU.mult, op1=ALU.mult, accum_out=var[:])
                  v = a_sm.tile([P, 1], DT_F32, tag="v")
                  nc.vector.tensor_scalar(
                      out=v[:], in0=var[:], scalar1=1.0 / E, scalar2=LN_EPS,
                      op0=ALU.mult, op1=ALU.add)
                  inv = a_sm.tile([P, 1], DT_F32, tag="inv")
                  if act_ln:
                      # tail-only: ACT is idle there, so ln+exp invstd beats
                      # the serial DVE Newton chain on latency
                      nc.scalar.activation(inv[:], var[:], AF.Ln,
                                           bias=eps_sb[:], scale=1.0 / E)
                      nc.scalar.activation(inv[:], inv[:], AF.Exp, scale=-0.5)
                  else:
                      nc.vector.reciprocal(inv[:], v[:])
                      # seed y0 = (1 + 1/v)/2: <7% err for v in [0.4, 2.5]
                      nc.vector.tensor_scalar(
                          out=inv[:], in0=inv[:], scalar1=0.5, scalar2=0.5,
                          op0=ALU.mult, op1=ALU.add)
                      t = a_sm.tile([P, 1], DT_F32, tag="nt")
                      for _ in range(2):
                          nc.vector.tensor_mul(t[:], inv[:], inv[:])
                          nc.vector.tensor_mul(t[:], t[:], v[:])
                          nc.vector.tensor_scalar(
                              out=t[:], in0=t[:], scalar1=-0.5, scalar2=1.5,
                              op0=ALU.mult, op1=ALU.add)
                          nc.vector.tensor_mul(inv[:], inv[:], t[:])
                  nc.vector.scalar_tensor_tensor(
                      out=out_ap, in0=xc[:], scalar=inv[:], in1=gb,
                      op0=ALU.mult, op1=ALU.mult)
                  nc.vector.tensor_add(out_ap, out_ap, bb)

              def wo_chunk(i, ts, act_ln=False):
                  wo_ps = mmps.tile([P, E], DT_F32, tag="mm")
                  for qt in range(KT):
                      nc.tensor.matmul(
                          wo_ps[:], att[:, i % 2, qt, ts * P:(ts + 1) * P],
                          CW['wos'][:, i, qt], start=(qt == 0), stop=(qt == KT - 1))
                  y1 = e_sb.tile([P, E], DT_F32, tag="y1")
                  nc.vector.tensor_add(y1[:], wo_ps[:], x32[:, i, ts])
                  layer_norm_into(r1[:, i, ts], y1, CW['g1b'][:, i],
                                  CW['b1b'][:, i], act_ln=act_ln)
                  tr = mmps.tile([P, E], DT_F32, tag="mm")
                  for et in range(ET):
                      nc.tensor.transpose(
                          tr[:, et * P:(et + 1) * P],
                          r1[:, i, ts, et * P:(et + 1) * P], identf[:])
                  nc.vector.tensor_copy(
                      r1T[:, i, :, ts * P:(ts + 1) * P],
                      tr[:].rearrange("p (a b) -> p a b", b=P))

              def ffn1_chunk(n, half):
                  # matmul then stage pre-activations to SBUF (DVE has
                  # slack); gelu runs later as one contiguous ACT block so
                  # the gelu<->exp table switches are bounded to one pair
                  # per stream.
                  w1s = w1_tiles[n][half]
                  if half == 0:
                      hp_t = hT_pool.tile([P, FT, TG], DT_BF, tag="hpre")
                      hT_tiles[n] = hp_t
                  else:
                      hp_t = hT_tiles[n]
                  for f2 in range(FT // 4):
                      h_ps = mmps.tile([P, E], DT_F32, tag="mm")
                      for s2 in range(2):
                          fs_l = f2 * 2 + s2
                          for et in range(ET):
                              nc.tensor.matmul(
                                  h_ps[:, s2 * TG:(s2 + 1) * TG],
                                  w1s[:, et, fs_l * P:(fs_l + 1) * P],
                                  r1T[:, n, et],
                                  start=(et == 0), stop=(et == ET - 1))
                      for s2 in range(2):
                          fs = half * (FT // 2) + f2 * 2 + s2
                          nc.vector.tensor_scalar_add(
                              hp_t[:, fs], h_ps[:, s2 * TG:(s2 + 1) * TG],
                              CW['bf1v'][:, n, fs:fs + 1])

              def gelu_chunk(n):
                  hp_t = hT_tiles[n]
                  hT = hT_pool.tile([P, FT, TG], DT_BF, tag="hT")
                  hT_tiles[n] = hT
                  for f4 in range(FT // 4):
                      nc.scalar.activation(
                          hT[:, f4 * 4:(f4 + 1) * 4],
                          hp_t[:, f4 * 4:(f4 + 1) * 4], AF.Gelu)

              def ffn2_chunk(n, ts, act_ln=False):
                  w2s = w2_tiles[n]
                  hT = hT_tiles[n]
                  f2_ps = mmps.tile([P, E], DT_F32, tag="mm")
                  for ft in range(FT):
                      nc.tensor.matmul(
                          f2_ps[:], hT[:, ft, ts * P:(ts + 1) * P],
                          w2s[:, ft], start=(ft == 0), stop=(ft == FT - 1))
                  y2 = e_sb.tile([P, E], DT_F32, tag="y1")
                  nc.vector.tensor_add(y2[:], f2_ps[:], CW['bf2b'][:, n])
                  nc.vector.tensor_add(y2[:], y2[:], r1[:, n, ts])
                  out_sb = e_sb.tile([P, E], DT_F32, tag="os")
                  layer_norm_into(out_sb[:], y2, CW['g2b'][:, n],
                                  CW['b2b'][:, n], act_ln=act_ln)
                  nc.sync.dma_start(out_d[n][:, ts], out_sb[:])
                  if ts == TS - 1:
                      del w2_tiles[n]
                      del hT_tiles[n]

              qk_proj(0)
              v_proj(0)
              for i in range(NS):
                  w1h0 = f_w1.tile([P, ET, 2 * E], DT_BF, tag="w1s")
                  w1h1 = f_w1.tile([P, ET, 2 * E], DT_BF, tag="w1s")
                  w1h = [w1h0, w1h1]
                  for h in range(2):
                      nc.sync.dma_start(
                          w1h[h][:], w1_d[i][:, :, h * 2 * E:(h + 1) * 2 * E])
                  w1_tiles[i] = w1h
                  if i >= 1:
                      w2s = f_w2.tile([P, FT, E], DT_BF, tag="w2s")
                      nc.sync.dma_start(w2s[:], w2_d[i - 1])
                      w2_tiles[i - 1] = w2s
                  if i == 0:
                      # stream 0's slots carry the remaining QKV projections
                      chunks = [
                          lambda: qk_proj(1), lambda: v_proj(1),
                          lambda: qk_proj(2), lambda: v_proj(2),
                          lambda: qk_proj(3), lambda: v_proj(3),
                      ]
                  else:
                      n = i - 1
                      chunks = [
                          lambda n=n: wo_chunk(n, 0),
                          lambda n=n: wo_chunk(n, 1),
                          lambda n=n: ffn1_chunk(n, 0),
                          lambda n=n: ffn1_chunk(n, 1),
                          lambda n=n: gelu_chunk(n),
                          lambda n=n: ffn2_chunk(n, 0),
                          lambda n=n: ffn2_chunk(n, 1),
                      ]
                      if i == 1:
                          # give setup_cw's DMAs two slots of headroom
                          chunks = [lambda: None, lambda: None] + chunks
                  ci = 0
                  for j in range(NS):
                      if i == 0 and j == 0:
                          pair_exs = emit_scores(0, 0)
                      exs = pair_exs
                      if ci < len(chunks):
                          chunks[ci]()
                          ci += 1
                      emit_uaT(i, j, exs)
                      # next pair's scores go out before chunk B so the exp
                      # stream never waits behind chunk matmuls
                      if j + 1 < NS:
                          pair_exs = emit_scores(i, j + 1)
                      elif i + 1 < NS:
                          pair_exs = emit_scores(i + 1, 0)
                      if ci < len(chunks):
                          chunks[ci]()
                          ci += 1
                  while ci < len(chunks):
                      chunks[ci]()
                      ci += 1
                  if i == 0:
                      p1w.release()
                      setup_cw()

              # tail: last stream's post-attention work, split by output-row
              # half (ts) so the two halves pipeline Wo->LN1->FFN1->gelu->FFN2
              def ffn1_ts(n, ts, hT):
                  w1h = w1_tiles[n]
                  for f4 in range(FT // 4):
                      h_ps = mmps.tile([P, E], DT_F32, tag="mm")
                      for s4 in range(4):
                          fs = f4 * 4 + s4
                          w1s = w1h[fs // (FT // 2)]
                          fs_l = fs % (FT // 2)
                          for et in range(ET):
                              nc.tensor.matmul(
                                  h_ps[:, s4 * P:(s4 + 1) * P],
                                  w1s[:, et, fs_l * P:(fs_l + 1) * P],
                                  r1T[:, n, et, ts * P:(ts + 1) * P],
                                  start=(et == 0), stop=(et == ET - 1))
                      for s4 in range(4):
                          fs = f4 * 4 + s4
                          nc.scalar.activation(
                              hT[:, fs, ts * P:(ts + 1) * P],
                              h_ps[:, s4 * P:(s4 + 1) * P],
                              AF.Gelu, bias=CW['bf1v'][:, n, fs:fs + 1])

              n = NS - 1
              w2s = f_w2.tile([P, FT, E], DT_BF, tag="w2s")
              nc.sync.dma_start(w2s[:], w2_d[n])
              w2_tiles[n] = w2s
              hT = hT_pool.tile([P, FT, TG], DT_BF, tag="hT")
              hT_tiles[n] = hT
              wo_chunk(n, 0)
              wo_chunk(n, 1)
              ffn1_ts(n, 0, hT)
              ffn1_ts(n, 1, hT)
              ffn2_chunk(n, 0)
              ffn2_chunk(n, 1)

              CW['pool'].release()
              hT_pool.release()
              sq_sb.release()
              e_sb.release()
              a_sm.release()
              a_sb.release()
              mmps.release()
              a_ups.release()
              a_sps.release()
              scopeA.release()
              scopeB.release()
              f_w2.release()
              f_w1.release()

    nc.compile()
    return nc


_NC_CACHE = {}


def _get_nc(reps=1, phases="all"):
    key = f"nc{reps}_{phases}"
    if key not in _NC_CACHE:
        _NC_CACHE[key] = _build_program(reps, phases)
    return _NC_CACHE[key]


def _pack_inputs(x0, x1, x2, x3, Wq, Wk, Wv, Wo, bo, ln1_g, ln1_b, ln2_g, ln2_b,
                 W1, bf1, W2, bf2, inter):
    x = np.stack([np.asarray(x0), np.asarray(x1), np.asarray(x2),
                  np.asarray(x3)]).astype(F32)  # [NS,B,S,E]
    Wq, Wk, Wv, Wo = (np.asarray(a, F32) for a in (Wq, Wk, Wv, Wo))
    inputs_bo = np.asarray(bo, F32)
    W1, W2 = np.asarray(W1, F32), np.asarray(W2, F32)
    inter = np.asarray(inter, F32)

    def tile_rows(a, nt):
        # [NS, R, C] -> [NS, P, nt, C]
        return np.ascontiguousarray(
            a.reshape(NS, nt, P, a.shape[-1]).transpose(0, 2, 1, 3))

    shared = {
        "wo": tile_rows(Wo / NS, ET).astype(BF16),
        "w1": tile_rows(W1, ET).astype(BF16),
        "w2": tile_rows(W2, FT).astype(BF16),
        "cmat": np.ascontiguousarray(
            np.broadcast_to((inter * SCALE).reshape(1, NS * NS), (P, NS * NS))
        ).astype(F32),
        "g1": np.ascontiguousarray(ln1_g, dtype=F32),
        "b1": np.ascontiguousarray(ln1_b, dtype=F32),
        "g2": np.ascontiguousarray(ln2_g, dtype=F32),
        "b2": np.ascontiguousarray(ln2_b, dtype=F32),
        "bf1": np.ascontiguousarray(
            np.asarray(bf1, F32).reshape(NS, FT, P).transpose(2, 0, 1)),
        "bf2": np.ascontiguousarray(bf2, dtype=F32),
    }
    per_hg = []
    for hg in range(HG):
        cols = slice(hg * HC * HD, (hg + 1) * HC * HD)
        per_hg.append({
            "wq": tile_rows(Wq[:, :, cols], ET).astype(BF16),
            "wk": tile_rows(Wk[:, :, cols], ET).astype(BF16),
            "wv": tile_rows(Wv[:, :, cols], ET).astype(BF16),
        })
    in_maps = []
    for core in range(N_CORES):
        b, hg = core // HG, core % HG
        xb = x[:, b]  # [NS, S, E]
        xT = np.ascontiguousarray(
            xb.transpose(0, 2, 1).reshape(NS, ET, P, S).transpose(0, 2, 1, 3)
        ).astype(BF16)
        x32 = np.ascontiguousarray(
            (xb[:, hg * TG:(hg + 1) * TG] + np.asarray(
                inputs_bo)[:, None, :]).reshape(NS, TS, P, E)
            .transpose(0, 2, 1, 3).astype(F32))
        m = {"xT": xT, "x32": x32}
        m.update(shared)
        m.update(per_hg[hg])
        in_maps.append(m)
    return in_maps


def _unpack_outputs(results):
    full = np.empty((NS, B, S, E), dtype=F32)
    for core in range(N_CORES):
        b, hg = core // HG, core % HG
        o = results[core]["out"]  # [NS, P, TS, E]
        full[:, b, hg * TG:(hg + 1) * TG] = (
            o.transpose(0, 2, 1, 3).reshape(NS, TG, E))
    return tuple(full[n] for n in range(NS))


def kernel(**inputs):
    nc = _get_nc()
    in_maps = _pack_inputs(**inputs)
    res = run_bass_kernel_spmd(
        nc, in_maps, core_ids=list(range(N_CORES)),
        trace=bool(int(os.environ.get("KERNEL_TRACE", "0"))))
    _NC_CACHE["last_result"] = res
    return _unpack_outputs(res.results)


def _bench_fn(inputs, reps=1, phases="all"):
    """Build a re-invocable jitted executable for the reps-replicated NEFF.

    Mirrors bass2jax.run_bass_via_pjrt's shard_map(_bass_exec) lowering but
    without output-buffer donation, so the same executable can be re-invoked
    and timed.
    """
    import jax
    from jax.sharding import Mesh, PartitionSpec, NamedSharding
    from jax.experimental.shard_map import shard_map
    from concourse import bass2jax
    from concourse import mybir as mb

    nc = _get_nc(reps, phases)
    bass2jax.install_neuronx_cc_hook()
    in_maps = _pack_inputs(**inputs)

    part_name = nc.partition_id_tensor.name if nc.partition_id_tensor else None
    in_names, out_names, out_avals, zero_outs = [], [], [], []
    for alloc in nc.m.functions[0].allocations:
        if not isinstance(alloc, mb.MemoryLocationSet):
            continue
        name = alloc.memorylocations[0].name
        if alloc.kind == "ExternalInput":
            if name != part_name:
                in_names.append(name)
        elif alloc.kind == "ExternalOutput":
            out_names.append(name)
            shape = tuple(alloc.tensor_shape)
            dtype = mb.dt.np(alloc.dtype)
            out_avals.append(jax.core.ShapedArray(shape, dtype))
            zero_outs.append(np.zeros(shape, dtype))
    n_params = len(in_names)
    all_names = in_names + out_names
    if part_name is not None:
        all_names = all_names + [part_name]

    def _body(*args):
        operands = list(args)
        if part_name is not None:
            operands.append(bass2jax.partition_id_tensor())
        outs = bass2jax._bass_exec_p.bind(
            *operands, out_avals=tuple(out_avals), in_names=tuple(all_names),
            out_names=tuple(out_names), lowering_input_output_aliases=(),
            sim_require_finite=True, sim_require_nnan=True, nc=nc)
        return tuple(outs)

    devices = jax.devices()[:N_CORES]
    mesh = Mesh(np.asarray(devices), ("core",))
    spec = PartitionSpec("core")
    fn = jax.jit(shard_map(
        _body, mesh=mesh, in_specs=(spec,) * (n_params + len(out_names)),
        out_specs=(spec,) * len(out_names), check_rep=False))
    sh = NamedSharding(mesh, spec)
    concat = [jax.device_put(
        np.concatenate([in_maps[c][nm] for c in range(N_CORES)], axis=0), sh)
        for nm in in_names]
    concat += [jax.device_put(
        np.zeros((N_CORES * z.shape[0], *z.shape[1:]), z.dtype), sh)
        for z in zero_outs]

    return fn, concat


def bench(inputs, iters=20, reps=1, phases="all"):
    """(min, median) wall seconds per call of the reps-replicated NEFF."""
    import time
    import jax

    fn, concat = _bench_fn(inputs, reps, phases)
    out = fn(*concat)  # compile
    jax.block_until_ready(out)
    times = []
    for _ in range(iters):
        t0 = time.perf_counter()
        out = fn(*concat)
        jax.block_until_ready(out)
        times.append(time.perf_counter() - t0)
    times.sort()
    return times[0], times[len(times) // 2]


def bench_interleaved(inputs, iters=30, reps=8):
    """Per-rep device time via interleaved reps=1 / reps=R calls.

    Alternating the two executables within each round and taking the median
    of per-round deltas cancels the axon-RPC overhead drift that breaks the
    separate-min slope estimate.
    """
    import time
    import jax

    fn1, args1 = _bench_fn(inputs, 1)
    fnR, argsR = _bench_fn(inputs, reps)
    jax.block_until_ready(fn1(*args1))
    jax.block_until_ready(fnR(*argsR))
    deltas = []
    for _ in range(iters):
        t0 = time.perf_counter()
        jax.block_until_ready(fn1(*args1))
        t1 = time.perf_counter()
        jax.block_until_ready(fnR(*argsR))
        t2 = time.perf_counter()
        deltas.append((t2 - t1) - (t1 - t0))
    deltas.sort()
    med = deltas[len(deltas) // 2]
    return med / (reps - 1), deltas


if __name__ == "__main__":
    import sys
    mode = sys.argv[1] if len(sys.argv) > 1 else "sim"
    sys.path.insert(0, os.path.dirname(os.path.abspath(__file__)))
    import reference

    inputs = {k: np.asarray(v) for k, v in reference.setup_inputs().items()}
    if mode == "sim":
        # Simulate core 0 (b=0, hg=0) with CoreSim and compare to reference.
        # CoreSim has no Gelu; patch exact erf-gelu into its activation visitor.
        import concourse.bass_interp as bass_interp
        from scipy.special import erf as _erf
        _orig_visit = bass_interp.InstructionExecutor.visit_InstActivation

        def _patched(self, instruction, reg_snapshot=None):
            if instruction.func == mybir.ActivationFunctionType.Gelu:
                instruction.func = mybir.ActivationFunctionType.Identity
                try:
                    import concourse.mybir as mb
                    from concourse.bass_interp import Direction
                    out_ap = instruction.outs[0]
                    res = _orig_visit(self, instruction, reg_snapshot=reg_snapshot)
                    v = self.view_ap(out_ap, Direction.WRITE, instruction,
                                     reg_snapshot=reg_snapshot)
                    x = v[:].astype(np.float32)
                    v[:] = (x * 0.5 * (1.0 + _erf(x / np.sqrt(2.0)))).astype(v.dtype)
                    return res
                finally:
                    instruction.func = mybir.ActivationFunctionType.Gelu
            return _orig_visit(self, instruction, reg_snapshot=reg_snapshot)

        bass_interp.InstructionExecutor.visit_InstActivation = _patched
        from concourse.bass_interp import CoreSim
        nc = _get_nc()
        in_maps = _pack_inputs(**inputs)
        sim = CoreSim(nc, trace=False)
        for name, arr in in_maps[0].items():
            sim.tensor(name)[:] = arr
        sim.simulate(check_with_hw=False)
        out = sim.tensor("out").copy()
        got = out.transpose(0, 2, 1, 3).reshape(NS, TG, E)
        exp = np.stack([np.asarray(o) for o in reference.reference(**inputs)])
        exp_slice = exp[:, 0, 0:TG]  # b=0, rows 0:256
        err = np.abs(got - exp_slice)
        rel = np.linalg.norm(got - exp_slice) / np.linalg.norm(exp_slice)
        print(f"max abs err: {err.max():.3e}  rel fro err: {rel:.3e}")
    else:
        got = kernel(**inputs)
        exp = reference.reference(**inputs)
        for n in range(NS):
            g, e = np.asarray(got[n]), np.asarray(exp[n])
            rel = np.linalg.norm(g - e) / np.linalg.norm(e)
            print(f"out{n}: rel fro err {rel:.3e} max abs {np.abs(g - e).max():.3e}")
